# revision 6
# baseline (speedup 1.0000x reference)
"""CrossAttention Trainium2 Bass kernel (8 NeuronCores).

Problem (fp32): x [4, 2048, 1024], y [4, 2048, 768]
  q = x@Wq + bq; k = y@Wk + bk; v = y@Wv + bv           (16 heads x d_head 64)
  out = softmax(q k^T / 8) v  reshaped, then @ Wo + bo  -> [4, 2048, 1024]

Sharding: 8 cores = 4 batches x 2 head-halves. Core c handles batch c//2 and
heads (c%2)*8 .. +8 (d-slice of 512) for the full sequence, producing a
partial output [2048, 1024] = attn_half @ Wo[d_slice, :]. Host sums the two
partials per batch and adds bo. No duplicated FLOPs, no collectives.

Design (all operand staging in bf16, f32 PSUM accumulation; measured rel err
~5e-3 vs the 2e-2 gate):
  - The kernel is ACT(exp)-bound: 33.5M exps/core ~ 255 us at 1 elem/lane/cyc.
    Everything else is software-pipelined under the exp stream.
  - scores: per head pair p, heads j=0/1 use PE row-tiling (K=64 at array
    rows 0-63 / 64-127) -> both heads' score matmuls run concurrently.
    One ACT exp instruction [128, 1024] covers both heads of one sk tile.
  - AV: col-tiled pair (M=64 at array cols 0-63 / 64-127) -> both heads
    accumulate into one [128, 512] PSUM tile concurrently.
  - softmax denominators: exp tiles are accumulated elementwise on DVE (bf16
    2x mode); one col-tiled ones-matmul pair broadcasts the partition-sums
    into a [128, 512] PSUM tile; reciprocal_approx_fast + one DVE multiply
    normalize straight into the bf16 attn staging tile.
  - Q/K/V/O projections + output DMA are emitted just-in-time between
    attention groups so PE/DVE/DMA fill the slack under ACT.
"""

import numpy as np
import ml_dtypes

import concourse.bass as bass
import concourse.mybir as mybir
import concourse.tile as tile
from concourse.bass_utils import run_bass_kernel_spmd

F32 = mybir.dt.float32
F32R = mybir.dt.float32r
BF16 = mybir.dt.bfloat16
AF = mybir.ActivationFunctionType
ALU = mybir.AluOpType

B, S, DE, DC = 4, 2048, 1024, 768
H, DH = 16, 64
HH = H // 2          # heads per core
DHALF = DE // 2      # 512, d-slice per core
NMT = DHALF // 128   # 4 head pairs
NKT_X = DE // 128    # 8 k-tiles for q projection
NKT_Y = DC // 128    # 6 k-tiles for k/v projections
NSK = S // 128       # 16 sk tiles
NMAC = S // 512      # 4 sq macros
SCALE = 1.0 / np.sqrt(DH)

_prog_cache = {}


def _split_sync_waits(nc):
    """This container's walrus accepts only 1 sync wait per instruction.
    Tile attaches one wait per producer proc. For every instruction with k>1
    waits, insert k-1 single-wait nops on the same engine right before it
    (equivalent semantics: the engine's stream waits serially)."""
    eng_map = {
        mybir.EngineType.PE: nc.tensor,
        mybir.EngineType.Activation: nc.scalar,
        mybir.EngineType.DVE: nc.vector,
        mybir.EngineType.Pool: nc.gpsimd,
        mybir.EngineType.SP: nc.sync,
    }
    for bb in nc.main_func.blocks:
        insts = bb.instructions
        fixes = []
        for idx, ins in enumerate(insts):
            si = ins.sync_info
            if si and si.on_wait and len(si.on_wait) > 1:
                fixes.append((idx, ins))
        for idx, ins in reversed(fixes):
            eng = eng_map.get(ins.engine)
            if eng is None:
                continue
            waits = list(ins.sync_info.on_wait)
            ins.sync_info.on_wait = [waits[-1]]
            nops = []
            for w in waits[:-1]:
                n = eng.nop(nofuse=True).ins
                for b2 in nc.main_func.blocks:
                    if b2.instructions and b2.instructions[-1] is n:
                        b2.instructions.pop()
                        break
                n.sync_info = mybir.SyncInfo(on_wait=[w], on_update=[])
                nops.append(n)
            for j, n in enumerate(nops):
                insts.insert(idx + j, n)
    return nc


def build_program(n_reps: int = 1):
    nc = bass.Bass()

    xs = nc.dram_tensor("xs", [NMAC * DE, 512], BF16, kind="ExternalInput")
    yT = nc.dram_tensor("yT", [DC, S], BF16, kind="ExternalInput")
    wq = nc.dram_tensor("wq", [DE, DHALF], BF16, kind="ExternalInput")
    wk = nc.dram_tensor("wk", [DC, DHALF], BF16, kind="ExternalInput")
    wv = nc.dram_tensor("wv", [DC, DHALF], BF16, kind="ExternalInput")
    wo = nc.dram_tensor("wo", [DHALF, DE], BF16, kind="ExternalInput")
    bqd = nc.dram_tensor("bq", [128, NMT], F32, kind="ExternalInput")
    bkd = nc.dram_tensor("bk", [128, NMT], F32, kind="ExternalInput")
    bvd = nc.dram_tensor("bv", [1, DHALF], BF16, kind="ExternalInput")
    onesd = nc.dram_tensor("ones", [1, 128], BF16, kind="ExternalInput")
    onescd = nc.dram_tensor("onesc", [128, 512], BF16, kind="ExternalInput")
    outd = nc.dram_tensor("out", [S, DE], F32, kind="ExternalOutput")

    from contextlib import ExitStack

    with tile.TileContext(nc) as tc:
      for _rep in range(n_reps):  # >1 only for timing (amortizes dispatch)
        with ExitStack() as ctx:
            pconst = ctx.enter_context(tc.tile_pool(name="const", bufs=1))
            ones_sb = pconst.tile([1, 128], BF16, name="ones")
            onesc = pconst.tile([128, 512], BF16, name="onesc")
            bv_sb = pconst.tile([1, DHALF], BF16, name="bv")
            bq_sb = pconst.tile([128, NMT], F32, name="bq")
            bk_sb = pconst.tile([128, NMT], F32, name="bk")
            warm_sb = pconst.tile([1, 128], BF16, name="warm")
            nc.sync.dma_start(ones_sb[:], onesd[:])
            nc.sync.dma_start(onesc[:], onescd[:])
            nc.sync.dma_start(bv_sb[:], bvd[:])
            nc.sync.dma_start(bq_sb[:], bqd[:])
            nc.sync.dma_start(bk_sb[:], bkd[:])
            # Load the exp table set during the DMA lead-in (one-time ~2.7us).
            nc.scalar.activation(warm_sb[:], ones_sb[:], AF.Exp, scale=1.0)

            # ---- input staging (bf16) ----
            pwk = ctx.enter_context(tc.tile_pool(name="wkp", bufs=NKT_Y))
            pyT = ctx.enter_context(tc.tile_pool(name="yTp", bufs=NKT_Y))
            pwq = ctx.enter_context(tc.tile_pool(name="wqp", bufs=NKT_X))
            pxs = ctx.enter_context(tc.tile_pool(name="xsp", bufs=NKT_X * NMAC))
            pwv = ctx.enter_context(tc.tile_pool(name="wvp", bufs=NKT_Y))
            pwo = ctx.enter_context(tc.tile_pool(name="wop", bufs=NMT))
            wkt, yt, wqt, wvt, wot = [], [], [], [], []
            xnn = [[None] * NKT_X for _ in range(NMAC)]
            for kt in range(NKT_Y):
                t = pwk.tile([128, DHALF], BF16, name="wkt")
                nc.sync.dma_start(t[:], wk[kt * 128 : (kt + 1) * 128, :])
                wkt.append(t)
            for kt in range(NKT_Y):
                t = pyT.tile([128, S], BF16, name="yt")
                nc.sync.dma_start(t[:], yT[kt * 128 : (kt + 1) * 128, :])
                yt.append(t)
            for kt in range(NKT_X):
                t = pwq.tile([128, DHALF], BF16, name="wqt")
                nc.sync.dma_start(t[:], wq[kt * 128 : (kt + 1) * 128, :])
                wqt.append(t)
            for kt in range(NKT_X):  # x slice for mac 0 first
                t = pxs.tile([128, 512], BF16, name="xt")
                nc.sync.dma_start(t[:], xs[kt * 128 : (kt + 1) * 128, :])
                xnn[0][kt] = t
            for kt in range(NKT_Y):
                t = pwv.tile([128, DHALF], BF16, name="wvt")
                nc.sync.dma_start(t[:], wv[kt * 128 : (kt + 1) * 128, :])
                wvt.append(t)
            for nn in range(1, NMAC):
                for kt in range(NKT_X):
                    t = pxs.tile([128, 512], BF16, name="xt")
                    lo = nn * DE + kt * 128
                    nc.sync.dma_start(t[:], xs[lo : lo + 128, :])
                    xnn[nn][kt] = t
            for kt in range(NMT):
                t = pwo.tile([128, DE], BF16, name="wot")
                nc.sync.dma_start(t[:], wo[kt * 128 : (kt + 1) * 128, :])
                wot.append(t)

            # ---- persistent staging ----
            pqT = ctx.enter_context(tc.tile_pool(name="qT", bufs=NMT))
            qT = [pqT.tile([128, S], BF16, name="qT") for _ in range(NMT)]
            pKT = ctx.enter_context(tc.tile_pool(name="KT", bufs=NMT))
            KT = [pKT.tile([128, S], BF16, name="KT") for _ in range(NMT)]
            pv = ctx.enter_context(tc.tile_pool(name="v", bufs=NSK))
            vsb = [pv.tile([128, DHALF], BF16, name="v") for _ in range(NSK)]
            pattn = ctx.enter_context(tc.tile_pool(name="attn", bufs=NMT))
            attn = [pattn.tile([128, S], BF16, name="attn") for _ in range(NMT)]

            # ---- PSUM pools: 4 + 2 + 2 = 8 banks ----
            pssc = ctx.enter_context(
                tc.tile_pool(name="scps", bufs=2, space="PSUM")
            )
            psav = ctx.enter_context(
                tc.tile_pool(name="avps", bufs=2, space="PSUM")
            )
            psaux = ctx.enter_context(
                tc.tile_pool(name="auxps", bufs=2, space="PSUM")
            )

            pexp = ctx.enter_context(tc.tile_pool(name="expp", bufs=6))
            pesum = ctx.enter_context(tc.tile_pool(name="esump", bufs=2))
            prcp = ctx.enter_context(tc.tile_pool(name="rcpp", bufs=2))
            posb = ctx.enter_context(tc.tile_pool(name="osbp", bufs=3))

            # ---- PE warm-up: keep HAM busy during the DMA lead-in ----
            for _ in range(40):
                wm = psaux.tile([64, 512], F32, name="warmps", tag="aux")
                nc.tensor.matmul(
                    wm[:], onesc[:, 0:64], onesc[:], start=True, stop=True
                )

            # ---- emit helpers ----
            def emit_kproj(mt, nn):
                ps = psaux.tile([128, 512], F32, name="kps", tag="aux")
                for kt in range(NKT_Y):
                    nc.tensor.matmul(
                        ps[:],
                        wkt[kt][:, mt * 128 : (mt + 1) * 128],
                        yt[kt][:, nn * 512 : (nn + 1) * 512],
                        start=(kt == 0),
                        stop=(kt == NKT_Y - 1),
                    )
                with nc.allow_low_precision(reason="bf16 store"):
                    nc.vector.tensor_scalar(
                        KT[mt][:, nn * 512 : (nn + 1) * 512],
                        ps[:],
                        bk_sb[:, mt : mt + 1],
                        None,
                        ALU.add,
                    )

            def emit_qproj(mt, nn):
                ps = psaux.tile([128, 512], F32, name="qps", tag="aux")
                for kt in range(NKT_X):
                    nc.tensor.matmul(
                        ps[:],
                        wqt[kt][:, mt * 128 : (mt + 1) * 128],
                        xnn[nn][kt][:],
                        start=(kt == 0),
                        stop=(kt == NKT_X - 1),
                    )
                with nc.allow_low_precision(reason="bf16 store"):
                    nc.vector.tensor_scalar(
                        qT[mt][:, nn * 512 : (nn + 1) * 512],
                        ps[:],
                        bq_sb[:, mt : mt + 1],
                        None,
                        ALU.add,
                    )

            def emit_vproj(t):
                ps = psaux.tile([128, 512], F32, name="vps", tag="aux")
                nc.tensor.matmul(
                    ps[:], ones_sb[:, :128], bv_sb[:], start=True, stop=False
                )
                for kt in range(NKT_Y):
                    nc.tensor.matmul(
                        ps[:],
                        yt[kt][:, t * 128 : (t + 1) * 128],
                        wvt[kt][:],
                        start=False,
                        stop=(kt == NKT_Y - 1),
                    )
                with nc.allow_low_precision(reason="bf16 store"):
                    nc.vector.tensor_copy(vsb[t][:], ps[:])

            def emit_oproj(mac, sm):
                smg = mac * 4 + sm
                osb = posb.tile([128, DE], F32, name="osb")
                for nnn in range(2):
                    lo = nnn * 512
                    ps = psaux.tile([128, 512], F32, name="ops", tag="aux")
                    for kt in range(NMT):
                        nc.tensor.matmul(
                            ps[:],
                            attn[kt][:, smg * 128 : (smg + 1) * 128],
                            wot[kt][:, lo : lo + 512],
                            start=(kt == 0),
                            stop=(kt == NMT - 1),
                        )
                    nc.vector.tensor_copy(osb[:, lo : lo + 512], ps[:])
                nc.sync.dma_start(outd[smg * 128 : (smg + 1) * 128, :], osb[:])

            # ---- lead-in compute ----
            for nn in range(NMAC):
                emit_kproj(0, nn)
            for nn in range(NMAC):
                emit_kproj(1, nn)
            emit_qproj(0, 0)
            for t in range(4):
                emit_vproj(t)

            # ---- attention, ACT-bound, everything else pipelined under it --
            def emit_scores(p, sq, t, sc):
                for j in range(2):
                    nc.tensor.matmul(
                        sc[:, j * 512 : (j + 1) * 512],
                        KT[p][j * 64 : j * 64 + 64, t * 128 : (t + 1) * 128],
                        qT[p][j * 64 : j * 64 + 64, sq : sq + 512],
                        start=True,
                        stop=True,
                    )

            for mac in range(NMAC):
                sq = mac * 512
                for p in range(NMT):
                    it = mac * NMT + p
                    # JIT work for later iterations, spread across the t-loop
                    jit = []
                    if it == 0:
                        jit += [(tt, lambda t=tt: emit_vproj(t + 4)) for tt in range(12)]
                    if mac == 0 and p in (1, 2):
                        # KT[2] during (0,1); KT[3] during (0,2)
                        jit += [
                            (2 + 4 * i, lambda mt=p + 1, nn=i: emit_kproj(mt, nn))
                            for i in range(4)
                        ]
                    # Q for the next iteration (lead-in covered (0,0))
                    nxt_p, nxt_mac = (p + 1, mac) if p < NMT - 1 else (0, mac + 1)
                    if nxt_mac < NMAC:
                        jit.append(
                            (13 if it == 0 else 4,
                             lambda mt=nxt_p, nn=nxt_mac: emit_qproj(mt, nn))
                        )
                    # previous mac's output projection, one row-tile per iter
                    if mac >= 1:
                        jit.append((8, lambda m=mac - 1, sm=p: emit_oproj(m, sm)))
                    jit_d = {}
                    for tt, fn in jit:
                        jit_d.setdefault(tt, []).append(fn)

                    av = psav.tile([128, 512], F32, name="avps")
                    esum = pesum.tile([128, 1024], BF16, name="esum")
                    prev = None  # (ex, t) pending AV+esum
                    for t in range(NSK):
                        sc = pssc.tile([128, 1024], F32, name="scps")
                        emit_scores(p, sq, t, sc)
                        ex = pexp.tile([128, 1024], BF16, name="expt")
                        nc.scalar.activation(ex[:], sc[:], AF.Exp, scale=SCALE)
                        if prev is not None:
                            pex, pt = prev
                            for j in range(2):
                                nc.tensor.matmul(
                                    av[j * 64 : (j + 1) * 64, :],
                                    vsb[pt][:, p * 128 + j * 64 : p * 128 + (j + 1) * 64],
                                    pex[:, j * 512 : (j + 1) * 512],
                                    start=(pt == 0),
                                    stop=(pt == NSK - 1),
                                )
                            with nc.allow_low_precision(reason="bf16 accum"):
                                if pt == 0:
                                    nc.vector.tensor_copy(esum[:], pex[:])
                                else:
                                    nc.vector.tensor_add(esum[:], esum[:], pex[:])
                        prev = (ex, t)
                        for fn in jit_d.get(t, []):
                            fn()
                    pex, pt = prev
                    for j in range(2):
                        nc.tensor.matmul(
                            av[j * 64 : (j + 1) * 64, :],
                            vsb[pt][:, p * 128 + j * 64 : p * 128 + (j + 1) * 64],
                            pex[:, j * 512 : (j + 1) * 512],
                            start=(pt == 0),
                            stop=(pt == NSK - 1),
                        )
                    with nc.allow_low_precision(reason="bf16 accum"):
                        nc.vector.tensor_add(esum[:], esum[:], pex[:])

                    # normalization: denom broadcast -> recip -> scale
                    nb = psaux.tile([128, 512], F32, name="nbps", tag="aux")
                    for j in range(2):
                        nc.tensor.matmul(
                            nb[j * 64 : (j + 1) * 64, :],
                            onesc[:, 0:64],
                            esum[:, j * 512 : (j + 1) * 512],
                            start=True,
                            stop=True,
                        )
                    rc = prcp.tile([128, 512], F32, name="rcp")
                    nc.vector.reciprocal(rc[:], nb[:])
                    with nc.allow_low_precision(reason="bf16 store"):
                        nc.vector.tensor_mul(
                            attn[p][:, sq : sq + 512], av[:], rc[:]
                        )

            # ---- tail: last mac's output projection ----
            for sm in range(4):
                emit_oproj(NMAC - 1, sm)

    return _split_sync_waits(nc)


def _host_prep(x, y, Wq, bq, Wk, bk, Wv, bv, Wo, bo):
    bf = ml_dtypes.bfloat16
    x = np.asarray(x, dtype=np.float32)
    y = np.asarray(y, dtype=np.float32)
    Wq = np.asarray(Wq, dtype=np.float32)
    Wk = np.asarray(Wk, dtype=np.float32)
    Wv = np.asarray(Wv, dtype=np.float32)
    Wo = np.asarray(Wo, dtype=np.float32)
    bq = np.asarray(bq, dtype=np.float32)
    bk = np.asarray(bk, dtype=np.float32)
    bv = np.asarray(bv, dtype=np.float32)
    ones = np.ones((1, 128), dtype=bf)
    onesc = np.ones((128, 512), dtype=bf)
    in_maps = []
    for c in range(8):
        b, hh = c // 2, c % 2
        dlo = hh * DHALF
        xT = np.ascontiguousarray(x[b].T)  # [DE, S]
        # x sequence-sliced: rows nn*DE + d, cols 512
        xs = np.ascontiguousarray(
            xT.reshape(DE, NMAC, 512).transpose(1, 0, 2).reshape(NMAC * DE, 512)
        ).astype(bf)
        in_maps.append(
            {
                "xs": xs,
                "yT": np.ascontiguousarray(y[b].T).astype(bf),
                "wq": np.ascontiguousarray(Wq[:, dlo : dlo + DHALF]).astype(bf),
                "wk": np.ascontiguousarray(Wk[:, dlo : dlo + DHALF]).astype(bf),
                "wv": np.ascontiguousarray(Wv[:, dlo : dlo + DHALF]).astype(bf),
                "wo": np.ascontiguousarray(Wo[dlo : dlo + DHALF, :]).astype(bf),
                "bq": np.ascontiguousarray(
                    bq[dlo : dlo + DHALF].reshape(NMT, 128).T
                ),
                "bk": np.ascontiguousarray(
                    bk[dlo : dlo + DHALF].reshape(NMT, 128).T
                ),
                "bv": bv[dlo : dlo + DHALF].reshape(1, DHALF).astype(bf),
                "ones": ones,
                "onesc": onesc,
            }
        )
    return in_maps


def kernel(x, y, Wq, bq, Wk, bk, Wv, bv, Wo, bo, _results_out=None, _trace=False):
    if "nc" not in _prog_cache:
        _prog_cache["nc"] = build_program()
    nc = _prog_cache["nc"]
    in_maps = _host_prep(x, y, Wq, bq, Wk, bk, Wv, bv, Wo, bo)
    res = run_bass_kernel_spmd(nc, in_maps, core_ids=list(range(8)), trace=_trace)
    if _results_out is not None:
        _results_out.append(res)
    bo = np.asarray(bo, dtype=np.float32)
    parts = [res.results[c]["out"] for c in range(8)]
    out = np.stack(
        [parts[2 * b].astype(np.float32) + parts[2 * b + 1] + bo for b in range(B)]
    )
    return out


# revision 8
# speedup vs baseline: 7.0007x; 7.0007x over previous
"""CrossAttention Trainium2 Bass kernel (8 NeuronCores).

Problem (fp32): x [4, 2048, 1024], y [4, 2048, 768]
  q = x@Wq + bq; k = y@Wk + bk; v = y@Wv + bv           (16 heads x d_head 64)
  out = softmax(q k^T / 8) v  reshaped, then @ Wo + bo  -> [4, 2048, 1024]

Sharding: 8 cores = 4 batches x 2 head-halves. Core c handles batch c//2 and
heads (c%2)*8 .. +8 (d-slice of 512) for the full sequence, producing a
partial output [2048, 1024] = attn_half @ Wo[d_slice, :]. Host sums the two
partials per batch and adds bo. No duplicated FLOPs, no collectives.

Design (all operand staging in bf16, f32 PSUM accumulation; measured rel err
~5e-3 vs the 2e-2 gate):
  - The kernel is ACT(exp)-bound: 33.5M exps/core ~ 255 us at 1 elem/lane/cyc.
    Everything else is software-pipelined under the exp stream.
  - scores: per head pair p, heads j=0/1 use PE row-tiling (K=64 at array
    rows 0-63 / 64-127) -> both heads' score matmuls run concurrently.
    One ACT exp instruction [128, 1024] covers both heads of one sk tile.
  - AV: col-tiled pair (M=64 at array cols 0-63 / 64-127) -> both heads
    accumulate into one [128, 512] PSUM tile concurrently.
  - softmax denominators: exp tiles are accumulated elementwise on DVE (bf16
    2x mode); one col-tiled ones-matmul pair broadcasts the partition-sums
    into a [128, 512] PSUM tile; reciprocal_approx_fast + one DVE multiply
    normalize straight into the bf16 attn staging tile.
  - Q/K/V/O projections + output DMA are emitted just-in-time between
    attention groups so PE/DVE/DMA fill the slack under ACT.
"""

import numpy as np
import ml_dtypes

import concourse.bass as bass
import concourse.mybir as mybir
import concourse.tile as tile
from concourse.bass_utils import run_bass_kernel_spmd

F32 = mybir.dt.float32
F32R = mybir.dt.float32r
BF16 = mybir.dt.bfloat16
AF = mybir.ActivationFunctionType
ALU = mybir.AluOpType

B, S, DE, DC = 4, 2048, 1024, 768
H, DH = 16, 64
HH = H // 2          # heads per core
DHALF = DE // 2      # 512, d-slice per core
NMT = DHALF // 128   # 4 head pairs
NKT_X = DE // 128    # 8 k-tiles for q projection
NKT_Y = DC // 128    # 6 k-tiles for k/v projections
NSK = S // 128       # 16 sk tiles
NMAC = S // 512      # 4 sq macros
SCALE = 1.0 / np.sqrt(DH)

_prog_cache = {}


def _split_sync_waits(nc):
    """This container's walrus accepts only 1 sync wait per instruction.
    Tile attaches one wait per producer proc. For every instruction with k>1
    waits, insert k-1 single-wait nops on the same engine right before it
    (equivalent semantics: the engine's stream waits serially)."""
    eng_map = {
        mybir.EngineType.PE: nc.tensor,
        mybir.EngineType.Activation: nc.scalar,
        mybir.EngineType.DVE: nc.vector,
        mybir.EngineType.Pool: nc.gpsimd,
        mybir.EngineType.SP: nc.sync,
    }
    for bb in nc.main_func.blocks:
        insts = bb.instructions
        fixes = []
        for idx, ins in enumerate(insts):
            si = ins.sync_info
            if si and si.on_wait and len(si.on_wait) > 1:
                fixes.append((idx, ins))
        for idx, ins in reversed(fixes):
            eng = eng_map.get(ins.engine)
            if eng is None:
                continue
            waits = list(ins.sync_info.on_wait)
            ins.sync_info.on_wait = [waits[-1]]
            nops = []
            for w in waits[:-1]:
                n = eng.nop(nofuse=True).ins
                for b2 in nc.main_func.blocks:
                    if b2.instructions and b2.instructions[-1] is n:
                        b2.instructions.pop()
                        break
                n.sync_info = mybir.SyncInfo(on_wait=[w], on_update=[])
                nops.append(n)
            for j, n in enumerate(nops):
                insts.insert(idx + j, n)
    return nc


def build_program(n_reps: int = 1, flags=()):
    flags = set(flags)
    nc = bass.Bass()

    xs = nc.dram_tensor("xs", [NMAC * DE, 512], BF16, kind="ExternalInput")
    yT = nc.dram_tensor("yT", [DC, S], BF16, kind="ExternalInput")
    wq = nc.dram_tensor("wq", [DE, DHALF], BF16, kind="ExternalInput")
    wk = nc.dram_tensor("wk", [DC, DHALF], BF16, kind="ExternalInput")
    wv = nc.dram_tensor("wv", [DC, DHALF], BF16, kind="ExternalInput")
    wo = nc.dram_tensor("wo", [DHALF, DE], BF16, kind="ExternalInput")
    bqd = nc.dram_tensor("bq", [128, NMT], F32, kind="ExternalInput")
    bkd = nc.dram_tensor("bk", [128, NMT], F32, kind="ExternalInput")
    bvd = nc.dram_tensor("bv", [1, DHALF], BF16, kind="ExternalInput")
    onesd = nc.dram_tensor("ones", [1, 128], BF16, kind="ExternalInput")
    onescd = nc.dram_tensor("onesc", [128, 512], BF16, kind="ExternalInput")
    outd = nc.dram_tensor("out", [S, DE], F32, kind="ExternalOutput")

    from contextlib import ExitStack

    with tile.TileContext(nc) as tc:
      for _rep in range(n_reps):  # >1 only for timing (amortizes dispatch)
        with ExitStack() as ctx:
            pconst = ctx.enter_context(tc.tile_pool(name="const", bufs=1))
            ones_sb = pconst.tile([1, 128], BF16, name="ones")
            onesc = pconst.tile([128, 512], BF16, name="onesc")
            bv_sb = pconst.tile([1, DHALF], BF16, name="bv")
            bq_sb = pconst.tile([128, NMT], F32, name="bq")
            bk_sb = pconst.tile([128, NMT], F32, name="bk")
            warm_sb = pconst.tile([1, 128], BF16, name="warm")
            nc.sync.dma_start(ones_sb[:], onesd[:])
            nc.sync.dma_start(onesc[:], onescd[:])
            nc.sync.dma_start(bv_sb[:], bvd[:])
            nc.sync.dma_start(bq_sb[:], bqd[:])
            nc.sync.dma_start(bk_sb[:], bkd[:])
            # Load the exp table set during the DMA lead-in (one-time ~2.7us).
            nc.scalar.activation(warm_sb[:], ones_sb[:], AF.Exp, scale=1.0)

            # ---- input staging (bf16) ----
            pwk = ctx.enter_context(tc.tile_pool(name="wkp", bufs=NKT_Y))
            pyT = ctx.enter_context(tc.tile_pool(name="yTp", bufs=NKT_Y))
            pwq = ctx.enter_context(tc.tile_pool(name="wqp", bufs=NKT_X))
            pxs = ctx.enter_context(tc.tile_pool(name="xsp", bufs=NKT_X * NMAC))
            pwv = ctx.enter_context(tc.tile_pool(name="wvp", bufs=NKT_Y))
            pwo = ctx.enter_context(tc.tile_pool(name="wop", bufs=NMT))
            wkt, yt, wqt, wvt, wot = [], [], [], [], []
            xnn = [[None] * NKT_X for _ in range(NMAC)]
            for kt in range(NKT_Y):
                t = pwk.tile([128, DHALF], BF16, name="wkt")
                nc.sync.dma_start(t[:], wk[kt * 128 : (kt + 1) * 128, :])
                wkt.append(t)
            for kt in range(NKT_Y):
                t = pyT.tile([128, S], BF16, name="yt")
                nc.sync.dma_start(t[:], yT[kt * 128 : (kt + 1) * 128, :])
                yt.append(t)
            for kt in range(NKT_X):
                t = pwq.tile([128, DHALF], BF16, name="wqt")
                nc.sync.dma_start(t[:], wq[kt * 128 : (kt + 1) * 128, :])
                wqt.append(t)
            for kt in range(NKT_X):  # x slice for mac 0 first
                t = pxs.tile([128, 512], BF16, name="xt")
                nc.sync.dma_start(t[:], xs[kt * 128 : (kt + 1) * 128, :])
                xnn[0][kt] = t
            for kt in range(NKT_Y):
                t = pwv.tile([128, DHALF], BF16, name="wvt")
                nc.sync.dma_start(t[:], wv[kt * 128 : (kt + 1) * 128, :])
                wvt.append(t)
            for nn in range(1, NMAC):
                for kt in range(NKT_X):
                    t = pxs.tile([128, 512], BF16, name="xt")
                    lo = nn * DE + kt * 128
                    nc.sync.dma_start(t[:], xs[lo : lo + 128, :])
                    xnn[nn][kt] = t
            for kt in range(NMT):
                t = pwo.tile([128, DE], BF16, name="wot")
                nc.sync.dma_start(t[:], wo[kt * 128 : (kt + 1) * 128, :])
                wot.append(t)

            # ---- persistent staging ----
            pqT = ctx.enter_context(tc.tile_pool(name="qT", bufs=NMT))
            qT = [pqT.tile([128, S], BF16, name="qT") for _ in range(NMT)]
            pKT = ctx.enter_context(tc.tile_pool(name="KT", bufs=NMT))
            KT = [pKT.tile([128, S], BF16, name="KT") for _ in range(NMT)]
            pv = ctx.enter_context(tc.tile_pool(name="v", bufs=NSK))
            vsb = [pv.tile([128, DHALF], BF16, name="v") for _ in range(NSK)]
            pattn = ctx.enter_context(tc.tile_pool(name="attn", bufs=NMT))
            attn = [pattn.tile([128, S], BF16, name="attn") for _ in range(NMT)]

            # ---- PSUM pools: 4 + 2 + 2 = 8 banks ----
            pssc = ctx.enter_context(
                tc.tile_pool(name="scps", bufs=2, space="PSUM")
            )
            psav = ctx.enter_context(
                tc.tile_pool(name="avps", bufs=2, space="PSUM")
            )
            psaux = ctx.enter_context(
                tc.tile_pool(name="auxps", bufs=2, space="PSUM")
            )

            pexp = ctx.enter_context(tc.tile_pool(name="expp", bufs=6))
            pesum = ctx.enter_context(tc.tile_pool(name="esump", bufs=2))
            prcp = ctx.enter_context(tc.tile_pool(name="rcpp", bufs=2))
            posb = ctx.enter_context(tc.tile_pool(name="osbp", bufs=3))

            # ---- PE warm-up: keep HAM busy during the DMA lead-in ----
            for _ in range(40):
                wm = psaux.tile([64, 512], F32, name="warmps", tag="aux")
                nc.tensor.matmul(
                    wm[:], onesc[:, 0:64], onesc[:], start=True, stop=True
                )

            # ---- emit helpers ----
            def emit_kproj(mt, nn):
                ps = psaux.tile([128, 512], F32, name="kps", tag="aux")
                for kt in range(NKT_Y):
                    nc.tensor.matmul(
                        ps[:],
                        wkt[kt][:, mt * 128 : (mt + 1) * 128],
                        yt[kt][:, nn * 512 : (nn + 1) * 512],
                        start=(kt == 0),
                        stop=(kt == NKT_Y - 1),
                    )
                with nc.allow_low_precision(reason="bf16 store"):
                    nc.vector.tensor_scalar(
                        KT[mt][:, nn * 512 : (nn + 1) * 512],
                        ps[:],
                        bk_sb[:, mt : mt + 1],
                        None,
                        ALU.add,
                    )

            def emit_qproj(mt, nn):
                ps = psaux.tile([128, 512], F32, name="qps", tag="aux")
                for kt in range(NKT_X):
                    nc.tensor.matmul(
                        ps[:],
                        wqt[kt][:, mt * 128 : (mt + 1) * 128],
                        xnn[nn][kt][:],
                        start=(kt == 0),
                        stop=(kt == NKT_X - 1),
                    )
                with nc.allow_low_precision(reason="bf16 store"):
                    nc.vector.tensor_scalar(
                        qT[mt][:, nn * 512 : (nn + 1) * 512],
                        ps[:],
                        bq_sb[:, mt : mt + 1],
                        None,
                        ALU.add,
                    )

            def emit_vproj(t):
                ps = psaux.tile([128, 512], F32, name="vps", tag="aux")
                nc.tensor.matmul(
                    ps[:], ones_sb[:, :128], bv_sb[:], start=True, stop=False
                )
                for kt in range(NKT_Y):
                    nc.tensor.matmul(
                        ps[:],
                        yt[kt][:, t * 128 : (t + 1) * 128],
                        wvt[kt][:],
                        start=False,
                        stop=(kt == NKT_Y - 1),
                    )
                with nc.allow_low_precision(reason="bf16 store"):
                    nc.vector.tensor_copy(vsb[t][:], ps[:])

            def emit_oproj(mac, sm):
                smg = mac * 4 + sm
                osb = posb.tile([128, DE], F32, name="osb")
                for nnn in range(2):
                    lo = nnn * 512
                    ps = psaux.tile([128, 512], F32, name="ops", tag="aux")
                    for kt in range(NMT):
                        nc.tensor.matmul(
                            ps[:],
                            attn[kt][:, smg * 128 : (smg + 1) * 128],
                            wot[kt][:, lo : lo + 512],
                            start=(kt == 0),
                            stop=(kt == NMT - 1),
                        )
                    nc.vector.tensor_copy(osb[:, lo : lo + 512], ps[:])
                nc.sync.dma_start(outd[smg * 128 : (smg + 1) * 128, :], osb[:])

            # ---- lead-in compute ----
            for nn in range(NMAC):
                emit_kproj(0, nn)
            for nn in range(NMAC):
                emit_kproj(1, nn)
            emit_qproj(0, 0)
            for t in range(4):
                emit_vproj(t)

            # ---- attention, ACT-bound, everything else pipelined under it --
            def emit_scores(p, sq, t, sc):
                for j in range(2):
                    nc.tensor.matmul(
                        sc[:, j * 512 : (j + 1) * 512],
                        KT[p][j * 64 : j * 64 + 64, t * 128 : (t + 1) * 128],
                        qT[p][j * 64 : j * 64 + 64, sq : sq + 512],
                        start=True,
                        stop=True,
                    )

            for mac in range(NMAC):
                sq = mac * 512
                for p in range(NMT):
                    it = mac * NMT + p
                    # JIT work for later iterations, spread across the t-loop
                    jit = []
                    if it == 0:
                        jit += [(tt, lambda t=tt: emit_vproj(t + 4)) for tt in range(12)]
                    if mac == 0 and p in (1, 2):
                        # KT[2] during (0,1); KT[3] during (0,2)
                        jit += [
                            (2 + 4 * i, lambda mt=p + 1, nn=i: emit_kproj(mt, nn))
                            for i in range(4)
                        ]
                    # Q for the next iteration (lead-in covered (0,0))
                    nxt_p, nxt_mac = (p + 1, mac) if p < NMT - 1 else (0, mac + 1)
                    if nxt_mac < NMAC:
                        jit.append(
                            (13 if it == 0 else 4,
                             lambda mt=nxt_p, nn=nxt_mac: emit_qproj(mt, nn))
                        )
                    # previous mac's output projection, one row-tile per iter
                    if mac >= 1:
                        jit.append((8, lambda m=mac - 1, sm=p: emit_oproj(m, sm)))
                    jit_d = {}
                    for tt, fn in jit:
                        jit_d.setdefault(tt, []).append(fn)

                    av = (None if "no_av" in flags
                          else psav.tile([128, 512], F32, name="avps"))
                    esum = (None if "no_esum" in flags
                            else pesum.tile([128, 1024], BF16, name="esum"))
                    prev = None  # (ex, t) pending AV+esum
                    for t in range(NSK):
                        sc = pssc.tile([128, 1024], F32, name="scps")
                        emit_scores(p, sq, t, sc)
                        ex = pexp.tile([128, 1024], BF16, name="expt")
                        nc.scalar.activation(ex[:], sc[:], AF.Exp, scale=SCALE)
                        if prev is not None:
                            pex, pt = prev
                            if "no_av" not in flags:
                                for j in range(2):
                                    nc.tensor.matmul(
                                        av[j * 64 : (j + 1) * 64, :],
                                        vsb[pt][:, p * 128 + j * 64 : p * 128 + (j + 1) * 64],
                                        pex[:, j * 512 : (j + 1) * 512],
                                        start=(pt == 0),
                                        stop=(pt == NSK - 1),
                                    )
                            if "no_esum" not in flags:
                                with nc.allow_low_precision(reason="bf16 accum"):
                                    if pt == 0:
                                        nc.vector.tensor_copy(esum[:], pex[:])
                                    else:
                                        nc.vector.tensor_add(esum[:], esum[:], pex[:])
                        prev = (ex, t)
                        for fn in jit_d.get(t, []):
                            fn()
                    pex, pt = prev
                    if "no_av" not in flags:
                        for j in range(2):
                            nc.tensor.matmul(
                                av[j * 64 : (j + 1) * 64, :],
                                vsb[pt][:, p * 128 + j * 64 : p * 128 + (j + 1) * 64],
                                pex[:, j * 512 : (j + 1) * 512],
                                start=(pt == 0),
                                stop=(pt == NSK - 1),
                            )
                    if "no_esum" not in flags:
                        with nc.allow_low_precision(reason="bf16 accum"):
                            nc.vector.tensor_add(esum[:], esum[:], pex[:])

                    # normalization: denom broadcast -> recip -> scale
                    if "no_norm" in flags:
                        src = av[:] if av is not None else pex[:, 0:512]
                        with nc.allow_low_precision(reason="bf16 store"):
                            nc.vector.tensor_copy(attn[p][:, sq : sq + 512], src)
                    else:
                        nb = psaux.tile([128, 512], F32, name="nbps", tag="aux")
                        if "no_esum" not in flags:
                            for j in range(2):
                                nc.tensor.matmul(
                                    nb[j * 64 : (j + 1) * 64, :],
                                    onesc[:, 0:64],
                                    esum[:, j * 512 : (j + 1) * 512],
                                    start=True,
                                    stop=True,
                                )
                        else:
                            nc.tensor.matmul(
                                nb[:], onesc[:, 0:128], onesc[:], start=True, stop=True
                            )
                        rc = prcp.tile([128, 512], F32, name="rcp")
                        if "no_recip" in flags:
                            nc.vector.tensor_copy(rc[:], nb[:])
                        else:
                            nc.vector.reciprocal(rc[:], nb[:])
                        with nc.allow_low_precision(reason="bf16 store"):
                            nc.vector.tensor_mul(
                                attn[p][:, sq : sq + 512], av[:], rc[:]
                            )

            # ---- tail: last mac's output projection ----
            for sm in range(4):
                emit_oproj(NMAC - 1, sm)

    return _split_sync_waits(nc)


def _host_prep(x, y, Wq, bq, Wk, bk, Wv, bv, Wo, bo):
    bf = ml_dtypes.bfloat16
    x = np.asarray(x, dtype=np.float32)
    y = np.asarray(y, dtype=np.float32)
    Wq = np.asarray(Wq, dtype=np.float32)
    Wk = np.asarray(Wk, dtype=np.float32)
    Wv = np.asarray(Wv, dtype=np.float32)
    Wo = np.asarray(Wo, dtype=np.float32)
    bq = np.asarray(bq, dtype=np.float32)
    bk = np.asarray(bk, dtype=np.float32)
    bv = np.asarray(bv, dtype=np.float32)
    ones = np.ones((1, 128), dtype=bf)
    onesc = np.ones((128, 512), dtype=bf)
    in_maps = []
    for c in range(8):
        b, hh = c // 2, c % 2
        dlo = hh * DHALF
        xT = np.ascontiguousarray(x[b].T)  # [DE, S]
        # x sequence-sliced: rows nn*DE + d, cols 512
        xs = np.ascontiguousarray(
            xT.reshape(DE, NMAC, 512).transpose(1, 0, 2).reshape(NMAC * DE, 512)
        ).astype(bf)
        in_maps.append(
            {
                "xs": xs,
                "yT": np.ascontiguousarray(y[b].T).astype(bf),
                "wq": np.ascontiguousarray(Wq[:, dlo : dlo + DHALF]).astype(bf),
                "wk": np.ascontiguousarray(Wk[:, dlo : dlo + DHALF]).astype(bf),
                "wv": np.ascontiguousarray(Wv[:, dlo : dlo + DHALF]).astype(bf),
                "wo": np.ascontiguousarray(Wo[dlo : dlo + DHALF, :]).astype(bf),
                "bq": np.ascontiguousarray(
                    bq[dlo : dlo + DHALF].reshape(NMT, 128).T
                ),
                "bk": np.ascontiguousarray(
                    bk[dlo : dlo + DHALF].reshape(NMT, 128).T
                ),
                "bv": bv[dlo : dlo + DHALF].reshape(1, DHALF).astype(bf),
                "ones": ones,
                "onesc": onesc,
            }
        )
    return in_maps


def kernel(x, y, Wq, bq, Wk, bk, Wv, bv, Wo, bo, _results_out=None, _trace=False):
    if "nc" not in _prog_cache:
        _prog_cache["nc"] = build_program()
    nc = _prog_cache["nc"]
    in_maps = _host_prep(x, y, Wq, bq, Wk, bk, Wv, bv, Wo, bo)
    res = run_bass_kernel_spmd(nc, in_maps, core_ids=list(range(8)), trace=_trace)
    if _results_out is not None:
        _results_out.append(res)
    bo = np.asarray(bo, dtype=np.float32)
    parts = [res.results[c]["out"] for c in range(8)]
    out = np.stack(
        [parts[2 * b].astype(np.float32) + parts[2 * b + 1] + bo for b in range(B)]
    )
    return out
